# revision 24
# baseline (speedup 1.0000x reference)
"""Sliding-window attention (w=256) on 8 TRN2 NeuronCores.

Problem: q,k,v [b=2, s=4096, h=8, d=64] fp32, each query attends keys within
+/-256. Sharding: b*h = 16 head-slices; each core takes 2 ADJACENT heads of one
batch so every DMA row is 512B-contiguous (full line rate).

Per-core algorithm (heads h0,h1; 16 chunks of 256 queries):
  - Load q,k per 512-col batch (HWDGE fp32), DVE-cast to bf16, PE-transpose
    128x128 blocks into qT2/kT2 [128(=2*64 d-stacked), 512]-tiles: partitions
    0:64 = head0's d, 64:128 = head1's d. kT2 padded by w=256 zero cols on
    both sides. Per-batch tiles give the scheduler precise deps, so the chunk
    loop starts as soon as its first batches land.
  - v cast to bf16 per head: v_ext [128, 36, 65] (s-tiles on partitions,
    padded 2 tiles each side, 65th column of ones -> softmax denominator
    falls out of the ctx matmul for free).
  - Scores TRANSPOSED: S^T[y, x] = k.q (K=d=64), bf16, PACKED psum layout
    [128, 1280] = [j0 x-lo | j1 | j2 | j3 | j4 | j5 x-hi] - the quarter
    tiles that no ctx matmul consumes are never computed nor exp'd. The two
    heads' matmuls are row-packed (tile_position (0,0)/(64,0)) so they run
    concurrently in the 128x128 array.
  - One exp per chunk-head on ACT with the 1/sqrt(d) scale folded in; band
    masking via two [128,128] triangle-mask multiplies per head after exp.
  - ctx[x, 65] = sum_j E_j[:, x-half].T @ v_ext[2c+j] (bf16), normalize by
    the ones-column sum directly from PSUM, DMA out.
"""

import numpy as np

import concourse.bass as bass
import concourse.bacc as bacc
import concourse.mybir as mybir
from concourse.tile import TileContext
from concourse.bass_utils import run_bass_kernel_spmd
from concourse.masks import make_identity

F32 = mybir.dt.float32
BF16 = mybir.dt.bfloat16

S = 4096
D = 64
W = 256
C = S // W  # 16 chunks
NT = S // 128  # 32 s-tiles
NB = NT // 4  # 8 transpose batches of 4 tiles

# packed E/psum layout, PSUM-bank aligned (bank = 512 fp32): bank0 = [j1|j2],
# bank1 = [j3|j4], half bank2 = [j0 x-lo | j5 x-hi]; every scores matmul's
# output stays inside one bank.
EW = 1280


def _eoff(j, xt):
    if j == 0:
        assert xt == 0
        return 1024
    if j == 5:
        assert xt == 1
        return 1152
    return 256 * (j - 1) + 128 * xt


_CACHE = {}


def build_nc(repeats=1, loop_n=0):
    nc = bacc.Bacc("TRN2", target_bir_lowering=False)
    q = nc.dram_tensor("q", [S, 128], F32, kind="ExternalInput")
    k = nc.dram_tensor("k", [S, 128], F32, kind="ExternalInput")
    v = nc.dram_tensor("v", [S, 128], F32, kind="ExternalInput")
    out = nc.dram_tensor("out", [S, 128], F32, kind="ExternalOutput")

    with TileContext(nc) as tc:
        with (
            tc.tile_pool(name="const", bufs=1) as constp,
            tc.tile_pool(name="big", bufs=1) as bigp,
            tc.tile_pool(name="stage", bufs=6) as stagep,
            tc.tile_pool(name="spsum", bufs=2, space="PSUM") as spsum,
            tc.tile_pool(name="xpsum", bufs=1, space="PSUM") as xpsum,
            tc.tile_pool(name="epool", bufs=4) as epool,
            tc.tile_pool(name="rpool", bufs=8) as rpool,
            tc.tile_pool(name="opool", bufs=3) as opool,
        ):
            # ---- constants ----
            ident = constp.tile([128, 128], BF16)
            make_identity(nc, ident)
            # triangle masks [128, 128]: tle keeps x <= p, tge keeps x >= p
            tle = constp.tile([128, 128], BF16, name="tle")
            tge = constp.tile([128, 128], BF16, name="tge")
            for t, cm in ((tle, 1), (tge, -1)):
                nc.gpsimd.memset(t, 1.0)
                nc.gpsimd.affine_select(
                    out=t, in_=t,
                    compare_op=mybir.AluOpType.is_ge,
                    fill=0.0, base=0,
                    pattern=[[-cm, 128]],
                    channel_multiplier=cm,
                )

            # warm the ACT exp table set during phase A (hides ~2.7us load)
            warm = constp.tile([128, 1], F32, name="warm")
            nc.vector.memset(warm, 0.0)
            nc.scalar.activation(warm, warm, mybir.ActivationFunctionType.Exp)

            # ---- persistent buffers: per-batch tiles for precise deps ----
            # first two batches are 2 s-tiles (256 cols) to shorten the
            # time-to-first-chunk; the rest are 4 s-tiles (512 cols)
            BATCHES = [(4 * b, 4) for b in range(NB)]
            qT = [bigp.tile([128, 128 * n], BF16, name=f"qT{b}")
                  for b, (_, n) in enumerate(BATCHES)]
            kT = [bigp.tile([128, 128 * n], BF16, name=f"kT{b}")
                  for b, (_, n) in enumerate(BATCHES)]
            TSTART = [128 * t0 for t0, _ in BATCHES]
            vext = [bigp.tile([128, NT, D + 1], BF16, name=f"vext{h}") for h in range(2)]

            def _bat(off):
                for b in range(len(BATCHES) - 1, -1, -1):
                    if TSTART[b] <= off:
                        return b, off - TSTART[b]
                raise AssertionError(off)

            def kslice(g):
                """kT2 view at padded-global col g, width 128 (in-range only)."""
                assert W <= g < W + S
                b, off = _bat(g - W)
                return kT[b][:, off:off + 128]

            def qslice(x0, w):
                b, off = _bat(x0)
                return qT[b][:, off:off + w]

            vr = v[:, :].rearrange("(t p) (h d) -> p t h d", p=128, h=2)
            for h in range(2):
                nc.vector.memset(vext[h][:, :, D:D + 1], 1.0)
                nc.gpsimd.dma_start(vext[h][:, :, 0:D], vr[:, :, h, :])

            qr = q[:, :].rearrange("(t p) f -> p t f", p=128)
            kr = k[:, :].rearrange("(t p) f -> p t f", p=128)

            def emit_batch(b):
                """Load q/k batch b, cast to bf16, transpose."""
                t0, n = BATCHES[b]
                for which, srcr, dst in (("q", qr, qT[b]), ("k", kr, kT[b])):
                    stf = stagep.tile([128, 4, 128], F32, tag="stf")
                    nc.sync.dma_start(stf[:, :n, :], srcr[:, t0:t0 + n, :])
                    stb = stagep.tile([128, 4, 128], BF16, tag="stb")
                    nc.vector.tensor_copy(stb[:, :n, :], stf[:, :n, :])
                    tp = xpsum.tile([128, 512], BF16, tag="xa" if which == "q" else "xb")
                    for i in range(n):
                        nc.tensor.transpose(tp[:, 128 * i:128 * (i + 1)], stb[:, i, :], ident)
                    nc.vector.tensor_copy(dst, tp[:, :128 * n])

            def emit_chunk(c):
                # consumed j range per xt (edge chunks read fewer tiles)
                jlo = [0, 1]
                jhi = [4, 5]
                if c == 0:
                    jlo = [2, 2]
                if c == C - 1:
                    jhi = [3, 3]
                sp = [spsum.tile([128, EW], F32, name=f"sp{h}", tag="sp")
                      for h in range(2)]
                # scores, both heads row-packed: S^T[y, x] for consumed y-tiles
                for j in range(min(jlo), max(jhi) + 1):
                    if j == 0:
                        xs, xw = 0, 128     # x-lo half only
                    elif j == 5:
                        xs, xw = 128, 128   # x-hi half only
                    else:
                        xs, xw = 0, 256
                    eo = _eoff(j, 1 if j == 5 else 0)
                    for h in range(2):
                        nc.tensor.matmul(
                            sp[h][:, eo:eo + xw],
                            lhsT=kslice(W * c + 128 * j)[64 * h:64 * h + 64, :],
                            rhs=qslice(W * c + xs, xw)[64 * h:64 * h + 64, :],
                            start=True, stop=True,
                            tile_position=(64 * h, 0),
                        )
                if c == 0:
                    espans = [(256, 1024), (1152, 1280)]
                elif c == C - 1:
                    espans = [(0, 768), (1024, 1152)]
                else:
                    espans = [(0, 1280)]
                ostage = opool.tile([128, 2, 128], F32)
                for h in range(2):
                    E = epool.tile([128, EW], BF16)
                    for e0, e1 in espans:
                        nc.scalar.activation(E[:, e0:e1], sp[h][:, e0:e1],
                                             mybir.ActivationFunctionType.Exp,
                                             scale=float(D) ** -0.5)
                    # band masks on the consumed partial tiles (E *= 0/1)
                    if c != 0:
                        nc.gpsimd.tensor_tensor(E[:, 1024:1152], E[:, 1024:1152], tle,
                                                mybir.AluOpType.mult)
                        nc.vector.tensor_tensor(E[:, 128:256], E[:, 128:256], tle,
                                                mybir.AluOpType.mult)
                    if c != C - 1:
                        nc.vector.tensor_tensor(E[:, 768:896], E[:, 768:896], tge,
                                                mybir.AluOpType.mult)
                        nc.gpsimd.tensor_tensor(E[:, 1152:1280], E[:, 1152:1280], tge,
                                                mybir.AluOpType.mult)
                    # ctx[x, 65] = sum_j E_j[:, x-half].T @ vext[2c+j]
                    for xt in range(2):
                        ctx = xpsum.tile([128, D + 1], F32, tag="xa" if xt == 0 else "xb")
                        js = list(range(jlo[xt], jhi[xt] + 1))
                        for j in js:
                            eo = _eoff(j, xt)
                            nc.tensor.matmul(
                                ctx,
                                lhsT=E[:, eo:eo + 128],
                                rhs=vext[h][:, 2 * c + j - 2, :],
                                start=(j == js[0]), stop=(j == js[-1]),
                            )
                        rc = rpool.tile([128, 1], F32)
                        nc.vector.reciprocal(rc, ctx[:, D:D + 1])
                        nc.vector.tensor_scalar_mul(
                            ostage[:, xt, 64 * h:64 * h + 64], ctx[:, 0:D], rc)
                nc.sync.dma_start(
                    out[:, :].rearrange("(n p) f -> p n f", p=128)[:, 2 * c:2 * c + 2, :],
                    ostage)

            def emit_all():
                # emit each batch, then all chunks whose q/k needs are met
                done = [0]

                def ready(c):
                    # max padded-global col any matmul of chunk c touches
                    kmax = min(W * c + 768, W + S)   # kslice bound (g - W)
                    qmax = W * c + 256
                    return max(kmax - W, qmax)

                nb = len(BATCHES)
                for b in range(nb):
                    emit_batch(b)
                    avail = TSTART[b] + 128 * BATCHES[b][1]
                    if b + 2 < nb:
                        while done[0] < C and ready(done[0]) + 512 <= avail:
                            emit_chunk(done[0])
                            done[0] += 1
                while done[0] < C:
                    emit_chunk(done[0])
                    done[0] += 1

            if loop_n:
                with tc.For_i(0, loop_n, 1):
                    emit_all()
            else:
                for _ in range(repeats):
                    emit_all()
    nc.compile()
    return nc


def kernel(q, k, v, w):
    assert int(w) == W
    if "nc" not in _CACHE:
        _CACHE["nc"] = build_nc()
    nc = _CACHE["nc"]
    in_maps = []
    for core in range(8):
        b = core // 4
        h0 = 2 * (core % 4)
        in_maps.append({
            "q": np.ascontiguousarray(q[b, :, h0:h0 + 2, :]).reshape(S, 128),
            "k": np.ascontiguousarray(k[b, :, h0:h0 + 2, :]).reshape(S, 128),
            "v": np.ascontiguousarray(v[b, :, h0:h0 + 2, :]).reshape(S, 128),
        })
    res = run_bass_kernel_spmd(nc, in_maps, core_ids=list(range(8)))
    out = np.empty((2, S, 8, D), np.float32)
    for core, om in enumerate(res.results):
        b = core // 4
        h0 = 2 * (core % 4)
        out[b, :, h0:h0 + 2, :] = om["out"].reshape(S, 2, D)
    return out


# revision 27
# speedup vs baseline: 1.1247x; 1.1247x over previous
"""Sliding-window attention (w=256) on 8 TRN2 NeuronCores.

Problem: q,k,v [b=2, s=4096, h=8, d=64] fp32, each query attends keys within
+/-256. Sharding: b*h = 16 head-slices; each core takes 2 ADJACENT heads of one
batch so every DMA row is 512B-contiguous (full line rate).

Per-core algorithm (heads h0,h1; 16 chunks of 256 queries):
  - Load q,k per 512-col batch (HWDGE fp32), DVE-cast to bf16, PE-transpose
    128x128 blocks into qT2/kT2 [128(=2*64 d-stacked), 512]-tiles: partitions
    0:64 = head0's d, 64:128 = head1's d. kT2 padded by w=256 zero cols on
    both sides. Per-batch tiles give the scheduler precise deps, so the chunk
    loop starts as soon as its first batches land.
  - v cast to bf16 per head: v_ext [128, 36, 65] (s-tiles on partitions,
    padded 2 tiles each side, 65th column of ones -> softmax denominator
    falls out of the ctx matmul for free).
  - Scores TRANSPOSED: S^T[y, x] = k.q (K=d=64), bf16, PACKED psum layout
    [128, 1280] = [j0 x-lo | j1 | j2 | j3 | j4 | j5 x-hi] - the quarter
    tiles that no ctx matmul consumes are never computed nor exp'd. The two
    heads' matmuls are row-packed (tile_position (0,0)/(64,0)) so they run
    concurrently in the 128x128 array.
  - One exp per chunk-head on ACT with the 1/sqrt(d) scale folded in; band
    masking via two [128,128] triangle-mask multiplies per head after exp.
  - ctx[x, 65] = sum_j E_j[:, x-half].T @ v_ext[2c+j] (bf16), normalize by
    the ones-column sum directly from PSUM, DMA out.
"""

import numpy as np

import concourse.bass as bass
import concourse.bacc as bacc
import concourse.mybir as mybir
from concourse.tile import TileContext
from concourse.bass_utils import run_bass_kernel_spmd
from concourse.masks import make_identity

F32 = mybir.dt.float32
BF16 = mybir.dt.bfloat16

S = 4096
D = 64
W = 256
C = S // W  # 16 chunks
NT = S // 128  # 32 s-tiles
NB = NT // 4  # 8 transpose batches of 4 tiles

# packed E/psum layout, PSUM-bank aligned (bank = 512 fp32): bank0 = [j1|j2],
# bank1 = [j3|j4], half bank2 = [j0 x-lo | j5 x-hi]; every scores matmul's
# output stays inside one bank.
EW = 1280


def _eoff(j, xt):
    if j == 0:
        assert xt == 0
        return 1024
    if j == 5:
        assert xt == 1
        return 1152
    return 256 * (j - 1) + 128 * xt


_CACHE = {}


def build_nc(repeats=1, loop_n=0):
    nc = bacc.Bacc("TRN2", target_bir_lowering=False)
    q = nc.dram_tensor("q", [S, 128], F32, kind="ExternalInput")
    k = nc.dram_tensor("k", [S, 128], F32, kind="ExternalInput")
    v = nc.dram_tensor("v", [S, 128], F32, kind="ExternalInput")
    out = nc.dram_tensor("out", [S, 128], F32, kind="ExternalOutput")

    with TileContext(nc) as tc:
        with (
            tc.tile_pool(name="const", bufs=1) as constp,
            tc.tile_pool(name="big", bufs=1) as bigp,
            tc.tile_pool(name="stage", bufs=6) as stagep,
            tc.tile_pool(name="spsum", bufs=2, space="PSUM") as spsum,
            tc.tile_pool(name="xpsum", bufs=1, space="PSUM") as xpsum,
            tc.tile_pool(name="epool", bufs=4) as epool,
            tc.tile_pool(name="rpool", bufs=8) as rpool,
            tc.tile_pool(name="opool", bufs=3) as opool,
        ):
            # ---- constants ----
            ident = constp.tile([128, 128], BF16)
            make_identity(nc, ident)
            # triangle masks [128, 128]: tle keeps x <= p, tge keeps x >= p
            tle = constp.tile([128, 128], BF16, name="tle")
            tge = constp.tile([128, 128], BF16, name="tge")
            for t, cm in ((tle, 1), (tge, -1)):
                nc.gpsimd.memset(t, 1.0)
                nc.gpsimd.affine_select(
                    out=t, in_=t,
                    compare_op=mybir.AluOpType.is_ge,
                    fill=0.0, base=0,
                    pattern=[[-cm, 128]],
                    channel_multiplier=cm,
                )

            # warm the ACT exp table set during phase A (hides ~2.7us load)
            warm = constp.tile([128, 1], F32, name="warm")
            nc.vector.memset(warm, 0.0)
            nc.scalar.activation(warm, warm, mybir.ActivationFunctionType.Exp)

            # ---- persistent buffers: per-batch tiles for precise deps ----
            # first two batches are 2 s-tiles (256 cols) to shorten the
            # time-to-first-chunk; the rest are 4 s-tiles (512 cols)
            BATCHES = [(4 * b, 4) for b in range(NB)]
            qT = [bigp.tile([128, 128 * n], BF16, name=f"qT{b}")
                  for b, (_, n) in enumerate(BATCHES)]
            kT = [bigp.tile([128, 128 * n], BF16, name=f"kT{b}")
                  for b, (_, n) in enumerate(BATCHES)]
            TSTART = [128 * t0 for t0, _ in BATCHES]
            vext = [bigp.tile([128, NT, D + 1], BF16, name=f"vext{h}") for h in range(2)]

            def _bat(off):
                for b in range(len(BATCHES) - 1, -1, -1):
                    if TSTART[b] <= off:
                        return b, off - TSTART[b]
                raise AssertionError(off)

            def kslice(g):
                """kT2 view at padded-global col g, width 128 (in-range only)."""
                assert W <= g < W + S
                b, off = _bat(g - W)
                return kT[b][:, off:off + 128]

            def qslice(x0, w):
                b, off = _bat(x0)
                return qT[b][:, off:off + w]

            vr = v[:, :].rearrange("(t p) (h d) -> p t h d", p=128, h=2)
            for h in range(2):
                nc.vector.memset(vext[h][:, :, D:D + 1], 1.0)
                nc.gpsimd.dma_start(vext[h][:, :, 0:D], vr[:, :, h, :])

            qr = q[:, :].rearrange("(t p) f -> p t f", p=128)
            kr = k[:, :].rearrange("(t p) f -> p t f", p=128)

            def emit_batch(b):
                """Load q/k batch b, cast to bf16, transpose."""
                t0, n = BATCHES[b]
                for which, srcr, dst in (("q", qr, qT[b]), ("k", kr, kT[b])):
                    stf = stagep.tile([128, 4, 128], F32, tag="stf")
                    nc.sync.dma_start(stf[:, :n, :], srcr[:, t0:t0 + n, :])
                    stb = stagep.tile([128, 4, 128], BF16, tag="stb")
                    nc.vector.tensor_copy(stb[:, :n, :], stf[:, :n, :])
                    tp = xpsum.tile([128, 512], BF16, tag="xa" if which == "q" else "xb")
                    for i in range(n):
                        nc.tensor.transpose(tp[:, 128 * i:128 * (i + 1)], stb[:, i, :], ident)
                    nc.vector.tensor_copy(dst, tp[:, :128 * n])

            def emit_chunk(c):
                # consumed j range per xt (edge chunks read fewer tiles)
                jlo = [0, 1]
                jhi = [4, 5]
                if c == 0:
                    jlo = [2, 2]
                if c == C - 1:
                    jhi = [3, 3]
                sp = [spsum.tile([128, EW], F32, name=f"sp{h}", tag="sp")
                      for h in range(2)]
                # scores, both heads row-packed: S^T[y, x] for consumed y-tiles
                for j in range(min(jlo), max(jhi) + 1):
                    if j == 0:
                        xs, xw = 0, 128     # x-lo half only
                    elif j == 5:
                        xs, xw = 128, 128   # x-hi half only
                    else:
                        xs, xw = 0, 256
                    eo = _eoff(j, 1 if j == 5 else 0)
                    for h in range(2):
                        nc.tensor.matmul(
                            sp[h][:, eo:eo + xw],
                            lhsT=kslice(W * c + 128 * j)[64 * h:64 * h + 64, :],
                            rhs=qslice(W * c + xs, xw)[64 * h:64 * h + 64, :],
                            start=True, stop=True,
                            tile_position=(64 * h, 0),
                        )
                if c == 0:
                    espans = [(256, 1024), (1152, 1280)]
                elif c == C - 1:
                    espans = [(0, 768), (1024, 1152)]
                else:
                    espans = [(0, 1280)]
                ostage = opool.tile([128, 2, 128], F32)
                for h in range(2):
                    E = epool.tile([128, EW], BF16)
                    for e0, e1 in espans:
                        nc.scalar.activation(E[:, e0:e1], sp[h][:, e0:e1],
                                             mybir.ActivationFunctionType.Exp,
                                             scale=float(D) ** -0.5)
                    # band masks on the consumed partial tiles (E *= 0/1)
                    if c != 0:
                        nc.gpsimd.tensor_tensor(E[:, 1024:1152], E[:, 1024:1152], tle,
                                                mybir.AluOpType.mult)
                        nc.vector.tensor_tensor(E[:, 128:256], E[:, 128:256], tle,
                                                mybir.AluOpType.mult)
                    if c != C - 1:
                        nc.vector.tensor_tensor(E[:, 768:896], E[:, 768:896], tge,
                                                mybir.AluOpType.mult)
                        nc.gpsimd.tensor_tensor(E[:, 1152:1280], E[:, 1152:1280], tge,
                                                mybir.AluOpType.mult)
                    # ctx[x, 65] = sum_j E_j[:, x-half].T @ vext[2c+j]
                    for xt in range(2):
                        ctx = xpsum.tile([128, D + 1], F32, tag="xa" if xt == 0 else "xb")
                        js = list(range(jlo[xt], jhi[xt] + 1))
                        for j in js:
                            eo = _eoff(j, xt)
                            nc.tensor.matmul(
                                ctx,
                                lhsT=E[:, eo:eo + 128],
                                rhs=vext[h][:, 2 * c + j - 2, :],
                                start=(j == js[0]), stop=(j == js[-1]),
                            )
                        rc = rpool.tile([128, 1], F32)
                        nc.vector.reciprocal(rc, ctx[:, D:D + 1])
                        nc.vector.tensor_scalar_mul(
                            ostage[:, xt, 64 * h:64 * h + 64], ctx[:, 0:D], rc)
                nc.sync.dma_start(
                    out[:, :].rearrange("(n p) f -> p n f", p=128)[:, 2 * c:2 * c + 2, :],
                    ostage)

            def emit_all():
                # emit each batch, then all chunks whose q/k needs are met
                done = [0]

                def ready(c):
                    # max padded-global col any matmul of chunk c touches
                    kmax = min(W * c + 768, W + S)   # kslice bound (g - W)
                    qmax = W * c + 256
                    return max(kmax - W, qmax)

                nb = len(BATCHES)
                for b in range(nb):
                    emit_batch(b)
                    avail = TSTART[b] + 128 * BATCHES[b][1]
                    if b + 2 < nb:
                        while done[0] < C and ready(done[0]) + 512 <= avail:
                            emit_chunk(done[0])
                            done[0] += 1
                while done[0] < C:
                    emit_chunk(done[0])
                    done[0] += 1

            if loop_n:
                with tc.For_i(0, loop_n, 1):
                    emit_all()
            else:
                for _ in range(repeats):
                    emit_all()
    nc.compile()
    return nc


def kernel(q, k, v, w):
    q = np.asarray(q, dtype=np.float32)
    k = np.asarray(k, dtype=np.float32)
    v = np.asarray(v, dtype=np.float32)
    assert int(w) == W
    if "nc" not in _CACHE:
        _CACHE["nc"] = build_nc()
    nc = _CACHE["nc"]
    in_maps = []
    for core in range(8):
        b = core // 4
        h0 = 2 * (core % 4)
        in_maps.append({
            "q": np.ascontiguousarray(q[b, :, h0:h0 + 2, :]).reshape(S, 128),
            "k": np.ascontiguousarray(k[b, :, h0:h0 + 2, :]).reshape(S, 128),
            "v": np.ascontiguousarray(v[b, :, h0:h0 + 2, :]).reshape(S, 128),
        })
    res = run_bass_kernel_spmd(nc, in_maps, core_ids=list(range(8)))
    out = np.empty((2, S, 8, D), np.float32)
    for core, om in enumerate(res.results):
        b = core // 4
        h0 = 2 * (core % 4)
        out[b, :, h0:h0 + 2, :] = om["out"].reshape(S, 2, D)
    return out


# revision 30
# speedup vs baseline: 1.4460x; 1.2857x over previous
"""Sliding-window attention (w=256) on 8 TRN2 NeuronCores.

Problem: q,k,v [b=2, s=4096, h=8, d=64] fp32, each query attends keys within
+/-256. Sharding: b*h = 16 head-slices; each core takes 2 ADJACENT heads of one
batch so every DMA row is 512B-contiguous (full line rate).

Per-core algorithm (heads h0,h1; 16 chunks of 256 queries):
  - Load q,k per 512-col batch (HWDGE fp32), DVE-cast to bf16, PE-transpose
    128x128 blocks into qT2/kT2 [128(=2*64 d-stacked), 512]-tiles: partitions
    0:64 = head0's d, 64:128 = head1's d. kT2 padded by w=256 zero cols on
    both sides. Per-batch tiles give the scheduler precise deps, so the chunk
    loop starts as soon as its first batches land.
  - v cast to bf16 per head: v_ext [128, 36, 65] (s-tiles on partitions,
    padded 2 tiles each side, 65th column of ones -> softmax denominator
    falls out of the ctx matmul for free).
  - Scores TRANSPOSED: S^T[y, x] = k.q (K=d=64), bf16, PACKED psum layout
    [128, 1280] = [j0 x-lo | j1 | j2 | j3 | j4 | j5 x-hi] - the quarter
    tiles that no ctx matmul consumes are never computed nor exp'd. The two
    heads' matmuls are row-packed (tile_position (0,0)/(64,0)) so they run
    concurrently in the 128x128 array.
  - One exp per chunk-head on ACT with the 1/sqrt(d) scale folded in; band
    masking via two [128,128] triangle-mask multiplies per head after exp.
  - ctx[x, 65] = sum_j E_j[:, x-half].T @ v_ext[2c+j] (bf16), normalize by
    the ones-column sum directly from PSUM, DMA out.
"""

import numpy as np

import concourse.bass as bass
import concourse.bacc as bacc
import concourse.mybir as mybir
from concourse.tile import TileContext
from concourse.bass_utils import run_bass_kernel_spmd
from concourse.masks import make_identity

F32 = mybir.dt.float32
BF16 = mybir.dt.bfloat16

S = 4096
D = 64
W = 256
C = S // W  # 16 chunks
NT = S // 128  # 32 s-tiles
NB = NT // 4  # 8 transpose batches of 4 tiles

# packed E/psum layout, PSUM-bank aligned (bank = 512 fp32): bank0 = [j1|j2],
# bank1 = [j3|j4], half bank2 = [j0 x-lo | j5 x-hi]; every scores matmul's
# output stays inside one bank.
EW = 1280


def _eoff(j, xt):
    if j == 0:
        assert xt == 0
        return 1024
    if j == 5:
        assert xt == 1
        return 1152
    return 256 * (j - 1) + 128 * xt


_CACHE = {}


def build_nc(repeats=1, loop_n=0):
    nc = bacc.Bacc("TRN2", target_bir_lowering=False)
    q = nc.dram_tensor("q", [S, 128], F32, kind="ExternalInput")
    k = nc.dram_tensor("k", [S, 128], F32, kind="ExternalInput")
    v = nc.dram_tensor("v", [S, 128], F32, kind="ExternalInput")
    out = nc.dram_tensor("out", [S, 128], F32, kind="ExternalOutput")

    with TileContext(nc) as tc:
        with (
            tc.tile_pool(name="const", bufs=1) as constp,
            tc.tile_pool(name="big", bufs=1) as bigp,
            tc.tile_pool(name="stage", bufs=6) as stagep,
            tc.tile_pool(name="spsum", bufs=2, space="PSUM") as spsum,
            tc.tile_pool(name="xpsum", bufs=1, space="PSUM") as xpsum,
            tc.tile_pool(name="epool", bufs=4) as epool,
            tc.tile_pool(name="rpool", bufs=8) as rpool,
            tc.tile_pool(name="opool", bufs=3) as opool,
        ):
            # ---- constants ----
            ident = constp.tile([128, 128], BF16)
            make_identity(nc, ident)
            # triangle masks [128, 128]: tle keeps x <= p, tge keeps x >= p
            tle = constp.tile([128, 128], BF16, name="tle")
            tge = constp.tile([128, 128], BF16, name="tge")
            for t, cm in ((tle, 1), (tge, -1)):
                nc.gpsimd.memset(t, 1.0)
                nc.gpsimd.affine_select(
                    out=t, in_=t,
                    compare_op=mybir.AluOpType.is_ge,
                    fill=0.0, base=0,
                    pattern=[[-cm, 128]],
                    channel_multiplier=cm,
                )

            # warm the ACT exp table set during phase A (hides ~2.7us load)
            warm = constp.tile([128, 1], F32, name="warm")
            nc.vector.memset(warm, 0.0)
            nc.scalar.activation(warm, warm, mybir.ActivationFunctionType.Exp)

            # ---- persistent buffers: per-batch tiles for precise deps ----
            BATCHES = [(4 * b, 4) for b in range(NB)]
            qT = [bigp.tile([128, 128 * n], BF16, name=f"qT{b}")
                  for b, (_, n) in enumerate(BATCHES)]
            kT = [bigp.tile([128, 128 * n], BF16, name=f"kT{b}")
                  for b, (_, n) in enumerate(BATCHES)]
            TSTART = [128 * t0 for t0, _ in BATCHES]
            vext = [bigp.tile([128, NT, D + 1], BF16, name=f"vext{h}") for h in range(2)]

            def _bat(off):
                for b in range(len(BATCHES) - 1, -1, -1):
                    if TSTART[b] <= off:
                        return b, off - TSTART[b]
                raise AssertionError(off)

            def kslice(g):
                """kT2 view at padded-global col g, width 128 (in-range only)."""
                assert W <= g < W + S
                b, off = _bat(g - W)
                return kT[b][:, off:off + 128]

            def qslice(x0, w):
                b, off = _bat(x0)
                return qT[b][:, off:off + w]

            vr = v[:, :].rearrange("(t p) (h d) -> p t h d", p=128, h=2)
            for h in range(2):
                nc.vector.memset(vext[h][:, :, D:D + 1], 1.0)
                nc.gpsimd.dma_start(vext[h][:, :, 0:D], vr[:, :, h, :])

            qr = q[:, :].rearrange("(t p) f -> p t f", p=128)
            kr = k[:, :].rearrange("(t p) f -> p t f", p=128)

            def emit_batch(b):
                """Load q/k batch b, cast to bf16, transpose. The first two
                batches pipeline at half-batch granularity to shorten the
                time-to-first-chunk chain."""
                t0, n = BATCHES[b]
                halves = 2 if b < 2 and n == 4 else 1
                for which, srcr, dst in (("q", qr, qT[b]), ("k", kr, kT[b])):
                    stf = stagep.tile([128, 4, 128], F32, tag="stf")
                    stb = stagep.tile([128, 4, 128], BF16, tag="stb")
                    tp = xpsum.tile([128, 512], BF16, tag="xa" if which == "q" else "xb")
                    m = n // halves
                    for hh in range(halves):
                        sl = slice(hh * m, hh * m + m)
                        nc.sync.dma_start(stf[:, sl, :], srcr[:, t0 + hh * m:t0 + hh * m + m, :])
                        nc.vector.tensor_copy(stb[:, sl, :], stf[:, sl, :])
                        for i in range(hh * m, hh * m + m):
                            nc.tensor.transpose(tp[:, 128 * i:128 * (i + 1)], stb[:, i, :], ident)
                        nc.vector.tensor_copy(dst[:, 128 * hh * m:128 * (hh * m + m)],
                                              tp[:, 128 * hh * m:128 * (hh * m + m)])

            def emit_chunk(c):
                # consumed j range per xt (edge chunks read fewer tiles)
                jlo = [0, 1]
                jhi = [4, 5]
                if c == 0:
                    jlo = [2, 2]
                if c == C - 1:
                    jhi = [3, 3]
                sp = [spsum.tile([128, EW], F32, name=f"sp{h}", tag="sp")
                      for h in range(2)]
                # scores, both heads row-packed: S^T[y, x] for consumed y-tiles
                for j in range(min(jlo), max(jhi) + 1):
                    if j == 0:
                        xs, xw = 0, 128     # x-lo half only
                    elif j == 5:
                        xs, xw = 128, 128   # x-hi half only
                    else:
                        xs, xw = 0, 256
                    eo = _eoff(j, 1 if j == 5 else 0)
                    for h in range(2):
                        nc.tensor.matmul(
                            sp[h][:, eo:eo + xw],
                            lhsT=kslice(W * c + 128 * j)[64 * h:64 * h + 64, :],
                            rhs=qslice(W * c + xs, xw)[64 * h:64 * h + 64, :],
                            start=True, stop=True,
                            tile_position=(64 * h, 0),
                        )
                if c == 0:
                    espans = [(256, 1024), (1152, 1280)]
                elif c == C - 1:
                    espans = [(0, 768), (1024, 1152)]
                else:
                    espans = [(0, 1280)]
                ostage = opool.tile([128, 2, 128], F32)
                for h in range(2):
                    E = epool.tile([128, EW], BF16)
                    for e0, e1 in espans:
                        nc.scalar.activation(E[:, e0:e1], sp[h][:, e0:e1],
                                             mybir.ActivationFunctionType.Exp,
                                             scale=float(D) ** -0.5)
                    # band masks on the consumed partial tiles (E *= 0/1)
                    if c != 0:
                        nc.gpsimd.tensor_tensor(E[:, 1024:1152], E[:, 1024:1152], tle,
                                                mybir.AluOpType.mult)
                        nc.vector.tensor_tensor(E[:, 128:256], E[:, 128:256], tle,
                                                mybir.AluOpType.mult)
                    if c != C - 1:
                        nc.vector.tensor_tensor(E[:, 768:896], E[:, 768:896], tge,
                                                mybir.AluOpType.mult)
                        nc.gpsimd.tensor_tensor(E[:, 1152:1280], E[:, 1152:1280], tge,
                                                mybir.AluOpType.mult)
                    # ctx[x, 65] = sum_j E_j[:, x-half].T @ vext[2c+j]
                    for xt in range(2):
                        ctx = xpsum.tile([128, D + 1], F32, tag="xa" if xt == 0 else "xb")
                        js = list(range(jlo[xt], jhi[xt] + 1))
                        for j in js:
                            eo = _eoff(j, xt)
                            nc.tensor.matmul(
                                ctx,
                                lhsT=E[:, eo:eo + 128],
                                rhs=vext[h][:, 2 * c + j - 2, :],
                                start=(j == js[0]), stop=(j == js[-1]),
                            )
                        rc = rpool.tile([128, 1], F32)
                        nc.vector.reciprocal(rc, ctx[:, D:D + 1])
                        nc.vector.tensor_scalar_mul(
                            ostage[:, xt, 64 * h:64 * h + 64], ctx[:, 0:D], rc)
                nc.sync.dma_start(
                    out[:, :].rearrange("(n p) f -> p n f", p=128)[:, 2 * c:2 * c + 2, :],
                    ostage)

            def emit_all():
                # emit each batch, then all chunks whose q/k needs are met
                done = [0]

                def ready(c):
                    # max padded-global col any matmul of chunk c touches
                    kmax = min(W * c + 768, W + S)   # kslice bound (g - W)
                    qmax = W * c + 256
                    return max(kmax - W, qmax)

                nb = len(BATCHES)
                for b in range(nb):
                    emit_batch(b)
                    avail = TSTART[b] + 128 * BATCHES[b][1]
                    if b + 2 < nb:
                        guard = 0 if done[0] < 2 else 512
                        while done[0] < C and ready(done[0]) + guard <= avail:
                            emit_chunk(done[0])
                            done[0] += 1
                while done[0] < C:
                    emit_chunk(done[0])
                    done[0] += 1

            if loop_n:
                with tc.For_i(0, loop_n, 1):
                    emit_all()
            else:
                for _ in range(repeats):
                    emit_all()
    nc.compile()
    return nc


def kernel(q, k, v, w):
    q = np.asarray(q, dtype=np.float32)
    k = np.asarray(k, dtype=np.float32)
    v = np.asarray(v, dtype=np.float32)
    assert int(w) == W
    if "nc" not in _CACHE:
        _CACHE["nc"] = build_nc()
    nc = _CACHE["nc"]
    in_maps = []
    for core in range(8):
        b = core // 4
        h0 = 2 * (core % 4)
        in_maps.append({
            "q": np.ascontiguousarray(q[b, :, h0:h0 + 2, :]).reshape(S, 128),
            "k": np.ascontiguousarray(k[b, :, h0:h0 + 2, :]).reshape(S, 128),
            "v": np.ascontiguousarray(v[b, :, h0:h0 + 2, :]).reshape(S, 128),
        })
    res = run_bass_kernel_spmd(nc, in_maps, core_ids=list(range(8)))
    out = np.empty((2, S, 8, D), np.float32)
    for core, om in enumerate(res.results):
        b = core // 4
        h0 = 2 * (core % 4)
        out[b, :, h0:h0 + 2, :] = om["out"].reshape(S, 2, D)
    return out
